# revision 13
# baseline (speedup 1.0000x reference)
"""Trainium2 Bass kernel for nn_AdaptiveMultiBoxLoss (SSD multibox distillation loss).

Data-parallel over the batch dim across 8 NeuronCores.  Each core computes
partial sums (smooth-L1 loc losses, CE conf losses with hard-negative mining
via a per-row binary-search threshold top-k) over its 8 batch rows; the host
sums the 8x16 partials and performs the final division by N.

Key device-side decompositions:
  loss_c = sum_pos(lse) - sum_all conf[p, ct_p] + sum_all conf[:,0]
           - sum_pos conf[:,0] + topk(lc_masked)
  (exploits that ~98% of priors are background so the CE gather is column 0;
   the true gather sum is a one-hot trace accumulated on the TensorEngine)
  topk per row: binary-search a threshold with exact counts
  (tensor_scalar is_gt + fused accumulate), then an exact correction pass.
"""

import os
import sys

sys.path.insert(0, "/opt/trn_rl_repo")

from contextlib import ExitStack

import numpy as np

import concourse.bass as bass
import concourse.bacc as bacc
import concourse.mybir as mybir
import concourse.tile as tile
from concourse.bass_utils import run_bass_kernel_spmd

F32 = mybir.dt.float32
BF16 = mybir.dt.bfloat16
I32 = mybir.dt.int32
ALU = mybir.AluOpType
ACT = mybir.ActivationFunctionType

# ---- problem geometry (hardcoded) ----
B, P, C = 64, 8732, 81
NCORES = 8
R = B // NCORES            # 8 batch rows per core
NT = 69                    # 128-prior tiles per row (68 full + 1x28)
TFULL, TREM = 68, 28
TCOL = R * NT              # 552 columns in row-tiled layout
NFB, FBT = 3, 23           # conf stream: 3 blocks/row x 23 tiles
FBF = FBT * C              # 1863
LTT, LTFULL, LTREM = 546, 545, 96   # loc flat tiling: 546 tiles of 128 rows
LF = LTT * 4               # 2184
NPART = 16
NE_CONST = 128 * LF        # every element of the padded loc tile contributes +1
NITER = 12                 # binary search iterations over [0, 16]
HI_INIT = 16.0

# partials columns
(COL_BT, COL_BS, COL_AT, COL_CT, COL_DT, COL_AS, COL_CS, COL_DS,
 COL_LT, COL_LS, COL_TKT, COL_TKS, COL_NP) = range(13)

STAGE = int(os.environ.get("K_STAGE", "9"))


def build_nc():
    nc = bacc.Bacc("TRN2", target_bir_lowering=False, debug=False,
                   num_devices=NCORES)

    conf_T = nc.declare_dram_parameter("conf_T", [R, P, C], F32, isOutput=False)
    conf_S = nc.declare_dram_parameter("conf_S", [R, P, C], F32, isOutput=False)
    loc_T = nc.declare_dram_parameter("loc_T", [R, P, 4], F32, isOutput=False)
    loc_S = nc.declare_dram_parameter("loc_S", [R, P, 4], F32, isOutput=False)
    loc_t = nc.declare_dram_parameter("loc_t", [R, P, 4], F32, isOutput=False)
    conf_t = nc.declare_dram_parameter("conf_t", [R, P], I32, isOutput=False)
    iota_p = nc.declare_dram_parameter("iota", [128, FBF], F32, isOutput=False)
    rowsel_p = nc.declare_dram_parameter("rowsel", [128, 8], F32, isOutput=False)
    selpos_p = nc.declare_dram_parameter("selpos", [8, 128], F32, isOutput=False)
    eye_p = nc.declare_dram_parameter("eye81", [81, 81], F32, isOutput=False)
    ones_p = nc.declare_dram_parameter("ones128", [128, 1], F32, isOutput=False)
    out_p = nc.declare_dram_parameter("out", [1, NPART], F32, isOutput=True)

    with tile.TileContext(nc) as tc, ExitStack() as ctx:
        cpool = ctx.enter_context(tc.tile_pool(name="consts", bufs=1))
        pers = ctx.enter_context(tc.tile_pool(name="pers", bufs=1))
        small = ctx.enter_context(tc.tile_pool(name="small", bufs=1))
        pool_cT = ctx.enter_context(tc.tile_pool(name="confT", bufs=2))
        pool_cS = ctx.enter_context(tc.tile_pool(name="confS", bufs=2))
        pool_eT = ctx.enter_context(tc.tile_pool(name="expT", bufs=2))
        pool_eS = ctx.enter_context(tc.tile_pool(name="expS", bufs=2))
        pool_eq = ctx.enter_context(tc.tile_pool(name="eq", bufs=2))
        psum = ctx.enter_context(tc.tile_pool(name="ps", bufs=2, space="PSUM"))
        pstr = ctx.enter_context(tc.tile_pool(name="tr", bufs=1, space="PSUM"))

        # ---- constants ----
        iota_sb = cpool.tile([128, FBF], F32)
        rowsel_sb = cpool.tile([128, 8], F32)
        selpos_sb = cpool.tile([8, 128], F32)
        eye_sb = cpool.tile([81, 81], F32)
        ones_sb = cpool.tile([128, 1], F32)
        nc.sync.dma_start(out=iota_sb[:, :], in_=iota_p.ap())
        nc.sync.dma_start(out=rowsel_sb[:, :], in_=rowsel_p.ap())
        nc.sync.dma_start(out=selpos_sb[:, :], in_=selpos_p.ap())
        nc.sync.dma_start(out=eye_sb[:, :], in_=eye_p.ap())
        nc.sync.dma_start(out=ones_sb[:, :], in_=ones_p.ap())

        # ---- persistent tensors ----
        ctf_i = pers.tile([128, TCOL], I32)
        ctf = pers.tile([128, TCOL], F32)
        posf = pers.tile([128, TCOL], F32)
        ominus = pers.tile([128, TCOL], F32)
        vmask = pers.tile([128, TCOL], F32)
        sumexp = {x: pers.tile([128, TCOL], F32, name=f"sumexp{x}") for x in "TS"}
        conf0 = {x: pers.tile([128, TCOL], F32, name=f"conf0{x}") for x in "TS"}
        lse = {x: pers.tile([128, TCOL], F32, name=f"lse{x}") for x in "TS"}
        lcm = {x: pers.tile([128, TCOL], F32, name=f"lcm{x}") for x in "TS"}
        lcms = {x: pers.tile([128, TCOL], F32, name=f"lcms{x}") for x in "TS"}
        partials = pers.tile([128, NPART], F32)
        sgnjunk = pers.tile([128, TCOL], F32)

        ctfl_i = pers.tile([128, LTT], I32)
        ctfl = pers.tile([128, LTT], F32)
        posml = pers.tile([128, LTT], F32)
        locsb = {n: pers.tile([128, LTT, 4], F32, name=f"loc{n}")
                 for n in ("T", "S", "t")}
        ld = pers.tile([128, LF], F32)
        lu = pers.tile([128, LF], F32)
        lc_ = pers.tile([128, LF], F32)
        lm = pers.tile([128, LF], F32)

        nc.gpsimd.memset(partials[:, :], 0.0)

        # ---- conf_t: row-tiled layout ----
        ctfi_v = ctf_i[:, :].rearrange("p (r t) -> p r t", r=R)
        nc.gpsimd.memset(ctfi_v[:, :, NT - 1], -1)
        for r in range(R):
            nc.sync.dma_start(
                out=ctf_i[:, r * NT:r * NT + TFULL],
                in_=conf_t.ap()[r, 0:TFULL * 128].rearrange("(t p) -> p t", p=128))
            nc.sync.dma_start(
                out=ctf_i[0:TREM, r * NT + TFULL:r * NT + TFULL + 1],
                in_=conf_t.ap()[r, TFULL * 128:P].unsqueeze(1))
        nc.vector.tensor_copy(out=ctf[:, :], in_=ctf_i[:, :])
        nc.vector.tensor_scalar(out=posf[:, :], in0=ctf[:, :], scalar1=0.5,
                                scalar2=None, op0=ALU.is_gt)
        nc.vector.tensor_scalar(out=vmask[:, :], in0=ctf[:, :], scalar1=-0.5,
                                scalar2=None, op0=ALU.is_gt)
        nc.vector.tensor_tensor(out=ominus[:, :], in0=vmask[:, :],
                                in1=posf[:, :], op=ALU.subtract)

        # num_pos per row -> k
        npp = small.tile([128, 8], F32)
        nc.vector.tensor_reduce(out=npp[:, :],
                                in_=posf[:, :].rearrange("p (r t) -> p r t", r=R),
                                axis=mybir.AxisListType.X, op=ALU.add)
        ps_np = psum.tile([8, 1], F32, tag="ps")
        nc.tensor.matmul(ps_np[:, :], lhsT=npp[:, :], rhs=ones_sb[:, :],
                         start=True, stop=True)
        np8 = small.tile([8, 1], F32)
        nc.vector.tensor_copy(out=np8[:, :], in_=ps_np[:, :])
        k8 = small.tile([8, 1], F32)
        nc.vector.tensor_scalar(out=k8[:, :], in0=np8[:, :], scalar1=3.0,
                                scalar2=float(P - 1), op0=ALU.mult, op1=ALU.min)
        nc.vector.tensor_copy(out=partials[0:8, COL_NP:COL_NP + 1], in_=np8[:, :])

        # ---- conf_t flat layout (for loc masking) ----
        ct_flat = conf_t.ap().rearrange("r p -> (r p)")
        nc.gpsimd.memset(ctfl_i[:, LTT - 1:LTT], -1)
        nc.sync.dma_start(
            out=ctfl_i[:, 0:LTFULL],
            in_=ct_flat[0:LTFULL * 128].rearrange("(t p) -> p t", p=128))
        nc.sync.dma_start(
            out=ctfl_i[0:LTREM, LTFULL:LTT],
            in_=ct_flat[LTFULL * 128:R * P].unsqueeze(1))
        nc.vector.tensor_copy(out=ctfl[:, :], in_=ctfl_i[:, :])
        nc.vector.tensor_scalar(out=posml[:, :], in0=ctfl[:, :], scalar1=0.5,
                                scalar2=None, op0=ALU.is_gt)

        # ---- loc DMAs ----
        for name, param in (("T", loc_T), ("S", loc_S), ("t", loc_t)):
            dst = locsb[name]
            flat = param.ap().rearrange("r p f -> (r p) f")
            nc.gpsimd.memset(dst[:, LTT - 1, :], 0.0)
            nc.sync.dma_start(
                out=dst[:, 0:LTFULL, :],
                in_=flat[0:LTFULL * 128, :].rearrange("(t p) f -> p t f", p=128))
            nc.sync.dma_start(
                out=dst[0:LTREM, LTFULL, :],
                in_=flat[LTFULL * 128:R * P, :])

        # ---- loc smooth-L1 (masked, sum) ----
        # per element: 0.5*min(u,1)^2 + max(u,1) - 1 with u = |loc - loc_t|*pos
        # masked/pad elements contribute exactly +1, subtracted as NE_CONST.
        if STAGE >= 2:
            posml4 = posml[:, :].unsqueeze(2).broadcast_to((128, LTT, 4))
            for x, col in (("T", COL_LT), ("S", COL_LS)):
                nc.vector.tensor_tensor(
                    out=ld[:, :],
                    in0=locsb[x][:, :, :].rearrange("p t f -> p (t f)"),
                    in1=locsb["t"][:, :, :].rearrange("p t f -> p (t f)"),
                    op=ALU.subtract)
                nc.vector.tensor_tensor(
                    out=lu[:, :].rearrange("p (t f) -> p t f", f=4),
                    in0=ld[:, :].rearrange("p (t f) -> p t f", f=4),
                    in1=posml4, op=ALU.mult)
                nc.scalar.activation(out=lu[:, :], in_=lu[:, :], func=ACT.Abs)
                nc.vector.tensor_scalar(out=lc_[:, :], in0=lu[:, :], scalar1=1.0,
                                        scalar2=None, op0=ALU.min)
                nc.vector.tensor_scalar(out=lm[:, :], in0=lu[:, :], scalar1=1.0,
                                        scalar2=None, op0=ALU.max)
                nc.scalar.activation(out=lc_[:, :], in_=lc_[:, :], func=ACT.Square,
                                     scale=float(1.0 / np.sqrt(2.0)))
                nc.vector.tensor_tensor(out=ld[:, :], in0=lc_[:, :],
                                        in1=lm[:, :], op=ALU.add)
                nc.vector.tensor_reduce(out=partials[:, col:col + 1],
                                        in_=ld[:, :],
                                        axis=mybir.AxisListType.X, op=ALU.add)

        # ---- conf streaming loop ----
        pstr_t = {x: pstr.tile([81, 81], F32, name=f"pstr{x}") for x in "TS"}
        if STAGE >= 3:
            nmm = {"T": 0, "S": 0}
            total_mm = R * NFB * FBT
            for r in range(R):
                for fb in range(NFB):
                    cb = r * NT + fb * FBT
                    pb = fb * FBT * 128
                    ct_fb = {"T": pool_cT.tile([128, FBT, C], F32, name="ctT"),
                             "S": pool_cS.tile([128, FBT, C], F32, name="ctS")}
                    ex_fb = {"T": pool_eT.tile([128, FBT, C], BF16, name="exT"),
                             "S": pool_eS.tile([128, FBT, C], BF16, name="exS")}
                    for x, param in (("T", conf_T), ("S", conf_S)):
                        t = ct_fb[x]
                        if fb < NFB - 1:
                            nc.sync.dma_start(
                                out=t[:, :, :],
                                in_=param.ap()[r, pb:pb + FBT * 128, :]
                                    .rearrange("(t p) c -> p t c", p=128))
                        else:
                            nc.gpsimd.memset(t[:, FBT - 1, :], 0.0)
                            nc.sync.dma_start(
                                out=t[:, 0:FBT - 1, :],
                                in_=param.ap()[r, pb:pb + (FBT - 1) * 128, :]
                                    .rearrange("(t p) c -> p t c", p=128))
                            nc.sync.dma_start(
                                out=t[0:TREM, FBT - 1, :],
                                in_=param.ap()[r, TFULL * 128:P, :])
                        nc.scalar.activation(out=ex_fb[x][:, :, :], in_=t[:, :, :],
                                             func=ACT.Exp)

                    eq_t = pool_eq.tile([128, FBT, C], F32, name="eqt")
                    ctb_view = ctf[:, cb:cb + FBT].unsqueeze(2).broadcast_to(
                        (128, FBT, C))
                    nc.vector.tensor_tensor(
                        out=eq_t[:, :, :],
                        in0=iota_sb[:, :].rearrange("p (t c) -> p t c", c=C),
                        in1=ctb_view, op=ALU.is_equal)

                    for x in "TS":
                        nc.vector.tensor_reduce(out=sumexp[x][:, cb:cb + FBT],
                                                in_=ex_fb[x][:, :, :],
                                                axis=mybir.AxisListType.X,
                                                op=ALU.add)
                        nc.vector.tensor_copy(out=conf0[x][:, cb:cb + FBT],
                                              in_=ct_fb[x][:, :, 0])
                        # one-hot trace: psum[m, c] += sum_p eq[p, m]*conf[p, c]
                        for t in range(FBT):
                            nc.tensor.matmul(pstr_t[x][:, :],
                                             lhsT=eq_t[:, t, :],
                                             rhs=ct_fb[x][:, t, :],
                                             start=(nmm[x] == 0),
                                             stop=(nmm[x] == total_mm - 1))
                            nmm[x] += 1

        # ---- per-tensor epilogue: lse, partial sums, lc_m ----
        if STAGE >= 4:
            for x, (colA, colCc, colD, colB) in (
                    ("T", (COL_AT, COL_CT, COL_DT, COL_BT)),
                    ("S", (COL_AS, COL_CS, COL_DS, COL_BS))):
                nc.scalar.activation(out=lse[x][:, :], in_=sumexp[x][:, :],
                                     func=ACT.Ln)
                # A = sum(lse * posf)
                nc.vector.tensor_tensor(out=sgnjunk[:, :], in0=lse[x][:, :],
                                        in1=posf[:, :], op=ALU.mult)
                nc.vector.tensor_reduce(out=partials[:, colA:colA + 1],
                                        in_=sgnjunk[:, :],
                                        axis=mybir.AxisListType.X, op=ALU.add)
                # C = sum conf0
                nc.vector.tensor_reduce(out=partials[:, colCc:colCc + 1],
                                        in_=conf0[x][:, :],
                                        axis=mybir.AxisListType.X, op=ALU.add)
                # D = sum conf0 * posf
                nc.vector.tensor_tensor(out=sgnjunk[:, :], in0=conf0[x][:, :],
                                        in1=posf[:, :], op=ALU.mult)
                nc.vector.tensor_reduce(out=partials[:, colD:colD + 1],
                                        in_=sgnjunk[:, :],
                                        axis=mybir.AxisListType.X, op=ALU.add)
                # B = trace(pstr): diag via eye mask
                nc.vector.tensor_tensor(out=sgnjunk[0:81, 0:81],
                                        in0=pstr_t[x][:, :], in1=eye_sb[:, :],
                                        op=ALU.mult)
                nc.vector.tensor_reduce(out=partials[0:81, colB:colB + 1],
                                        in_=sgnjunk[0:81, 0:81],
                                        axis=mybir.AxisListType.X, op=ALU.add)
                # lc0 = lse - conf0 (into sumexp, which is dead)
                nc.vector.scalar_tensor_tensor(out=sumexp[x][:, :],
                                               in0=conf0[x][:, :], scalar=-1.0,
                                               in1=lse[x][:, :],
                                               op0=ALU.mult, op1=ALU.add)
                # lcm = lc0 * (valid - posf)   (zero at positives and pads)
                nc.vector.tensor_tensor(out=lcm[x][:, :], in0=sumexp[x][:, :],
                                        in1=ominus[:, :], op=ALU.mult)
                # reshuffle to search layout:
                # lcms[16r+l, ph*NT+t] = lcm[16ph+l, r*NT+t]
                for r in range(R):
                    for ph in range(8):
                        nc.sync.dma_start(
                            out=lcms[x][16 * r:16 * r + 16,
                                        ph * NT:(ph + 1) * NT],
                            in_=lcm[x][16 * ph:16 * ph + 16,
                                       r * NT:(r + 1) * NT])

        # ---- binary search for per-row top-k count thresholds ----
        lo = {x: small.tile([8, 1], F32, name=f"lo{x}") for x in "TS"}
        hi = {x: small.tile([8, 1], F32, name=f"hi{x}") for x in "TS"}
        tmid = {x: small.tile([8, 1], F32, name=f"tm{x}") for x in "TS"}
        ge = {x: small.tile([8, 1], I32, name=f"ge{x}") for x in "TS"}
        gei = {x: small.tile([8, 1], I32, name=f"gei{x}") for x in "TS"}
        s8 = {x: small.tile([8, 1], F32, name=f"s8{x}") for x in "TS"}
        pthr = {x: small.tile([128, 1], F32, name=f"pthr{x}") for x in "TS"}
        scnt = {x: small.tile([128, 1], F32, name=f"scnt{x}") for x in "TS"}
        ns = {x: small.tile([128, 2], F32, name=f"ns{x}") for x in "TS"}
        ns8 = {x: small.tile([8, 2], F32, name=f"ns8{x}") for x in "TS"}
        tk = {x: small.tile([8, 1], F32, name=f"tk{x}") for x in "TS"}
        if STAGE >= 5:
            for x in "TS":
                nc.gpsimd.memset(lo[x][:, :], 0.0)
                nc.gpsimd.memset(hi[x][:, :], HI_INIT)
            for it in range(NITER):
                for x in "TS":
                    nc.vector.tensor_tensor(out=tmid[x][:, :], in0=lo[x][:, :],
                                            in1=hi[x][:, :], op=ALU.add)
                    nc.vector.tensor_scalar(out=tmid[x][:, :], in0=tmid[x][:, :],
                                            scalar1=0.5, scalar2=None,
                                            op0=ALU.mult)
                    psA = psum.tile([128, 1], F32, name="psA", tag="ps")
                    nc.tensor.matmul(psA[:, :], lhsT=selpos_sb[:, :],
                                     rhs=tmid[x][:, :], start=True, stop=True)
                    nc.vector.tensor_copy(out=pthr[x][:, :], in_=psA[:, :])
                    nc.vector.tensor_scalar(out=sgnjunk[:, :], in0=lcms[x][:, :],
                                            scalar1=pthr[x][:, :], scalar2=None,
                                            op0=ALU.is_gt, op1=ALU.add,
                                            accum_out=scnt[x][:, :])
                    psB = psum.tile([8, 1], F32, name="psB", tag="ps")
                    nc.tensor.matmul(psB[:, :], lhsT=rowsel_sb[:, :],
                                     rhs=scnt[x][:, :], start=True, stop=True)
                    nc.vector.tensor_copy(out=s8[x][:, :], in_=psB[:, :])
                    nc.vector.tensor_tensor(out=ge[x][:, :], in0=s8[x][:, :],
                                            in1=k8[:, :], op=ALU.is_ge)
                    nc.vector.copy_predicated(out=lo[x][:, :], mask=ge[x][:, :],
                                              data=tmid[x][:, :])
                    nc.vector.tensor_scalar(out=gei[x][:, :], in0=ge[x][:, :],
                                            scalar1=1, scalar2=None,
                                            op0=ALU.bitwise_xor)
                    nc.vector.copy_predicated(out=hi[x][:, :], mask=gei[x][:, :],
                                              data=tmid[x][:, :])

        # ---- exact pass at t* = lo ----
        if STAGE >= 6:
            for x, colk in (("T", COL_TKT), ("S", COL_TKS)):
                psA = psum.tile([128, 1], F32, name="psA", tag="ps")
                nc.tensor.matmul(psA[:, :], lhsT=selpos_sb[:, :],
                                 rhs=lo[x][:, :], start=True, stop=True)
                nc.vector.tensor_copy(out=pthr[x][:, :], in_=psA[:, :])
                nc.vector.tensor_scalar(out=sgnjunk[:, :], in0=lcms[x][:, :],
                                        scalar1=pthr[x][:, :], scalar2=None,
                                        op0=ALU.is_gt, op1=ALU.add,
                                        accum_out=ns[x][:, 0:1])
                nc.vector.tensor_tensor(out=lse[x][:, :], in0=lcms[x][:, :],
                                        in1=sgnjunk[:, :], op=ALU.mult)
                nc.vector.tensor_reduce(out=ns[x][:, 1:2], in_=lse[x][:, :],
                                        axis=mybir.AxisListType.X, op=ALU.add)
                psC = psum.tile([8, 2], F32, name="psC", tag="ps")
                nc.tensor.matmul(psC[:, :], lhsT=rowsel_sb[:, :],
                                 rhs=ns[x][:, :], start=True, stop=True)
                nc.vector.tensor_copy(out=ns8[x][:, :], in_=psC[:, :])
                # topk = S* + (k - n*) * t*
                nc.vector.tensor_tensor(out=tk[x][:, :], in0=k8[:, :],
                                        in1=ns8[x][:, 0:1], op=ALU.subtract)
                nc.vector.tensor_tensor(out=tk[x][:, :], in0=tk[x][:, :],
                                        in1=lo[x][:, :], op=ALU.mult)
                nc.vector.tensor_tensor(out=tk[x][:, :], in0=tk[x][:, :],
                                        in1=ns8[x][:, 1:2], op=ALU.add)
                nc.vector.tensor_copy(out=partials[0:8, colk:colk + 1],
                                      in_=tk[x][:, :])

        # ---- final partition reduce of partials -> out ----
        psF = psum.tile([1, NPART], F32, name="psF", tag="ps")
        nc.tensor.matmul(psF[:, :], lhsT=ones_sb[:, :], rhs=partials[:, :],
                         start=True, stop=True)
        fin = small.tile([1, NPART], F32)
        nc.vector.tensor_copy(out=fin[:, :], in_=psF[:, :])
        nc.sync.dma_start(out=out_p.ap(), in_=fin[:, :])
    nc.finalize()
    return nc


_NC_CACHE = None


def _get_nc():
    global _NC_CACHE
    if _NC_CACHE is None:
        _NC_CACHE = build_nc()
    return _NC_CACHE


def _host_consts():
    iota = np.ascontiguousarray(
        np.tile(np.arange(C, dtype=np.float32), FBT)[None, :].repeat(128, 0))
    q = np.arange(128)
    rowsel = np.ascontiguousarray(
        (q[:, None] // 16 == np.arange(8)[None, :]).astype(np.float32))
    selpos = np.ascontiguousarray(
        (np.arange(8)[:, None] == q[None, :] // 16).astype(np.float32))
    eye81 = np.eye(81, dtype=np.float32)
    ones = np.ones((128, 1), np.float32)
    return iota, rowsel, selpos, eye81, ones


def _build_in_maps(inputs):
    conf_T = np.ascontiguousarray(np.asarray(inputs["conf_dataT"], np.float32))
    conf_S = np.ascontiguousarray(np.asarray(inputs["conf_dataS"], np.float32))
    loc_T = np.ascontiguousarray(np.asarray(inputs["loc_dataT"], np.float32))
    loc_S = np.ascontiguousarray(np.asarray(inputs["loc_dataS"], np.float32))
    loc_t = np.ascontiguousarray(np.asarray(inputs["loc_t"], np.float32))
    ct = np.ascontiguousarray(np.asarray(inputs["conf_t"], np.int32))
    iota, rowsel, selpos, eye81, ones = _host_consts()
    in_maps = []
    for d in range(NCORES):
        sl = slice(d * R, (d + 1) * R)
        in_maps.append({
            "conf_T": conf_T[sl], "conf_S": conf_S[sl],
            "loc_T": loc_T[sl], "loc_S": loc_S[sl], "loc_t": loc_t[sl],
            "conf_t": ct[sl],
            "iota": iota, "rowsel": rowsel, "selpos": selpos,
            "eye81": eye81, "ones128": ones,
        })
    return in_maps


def _combine(parts):
    S = parts.astype(np.float64).sum(axis=0)
    loss_cT = S[COL_AT] - S[COL_BT] + S[COL_CT] - S[COL_DT] + S[COL_TKT]
    loss_cS = S[COL_AS] - S[COL_BS] + S[COL_CS] - S[COL_DS] + S[COL_TKS]
    loss_lT = S[COL_LT] - NCORES * NE_CONST
    loss_lS = S[COL_LS] - NCORES * NE_CONST
    N = S[COL_NP]
    return np.array([loss_lT / N, loss_cT / N, loss_lS / N, loss_cS / N],
                    np.float32)


def run_on_hw(inputs, trace=False, **kw):
    nc = _get_nc()
    in_maps = _build_in_maps(inputs)
    res = run_bass_kernel_spmd(nc, in_maps, core_ids=list(range(NCORES)),
                               trace=trace, **kw)
    parts = np.stack([np.asarray(r["out"]).reshape(NPART) for r in res.results])
    return _combine(parts), res


def kernel(**inputs) -> np.ndarray:
    out, _ = run_on_hw(inputs, trace=False)
    return out
